# revision 23
# baseline (speedup 1.0000x reference)
"""Trainium2 Bass kernel for BasicMoE.

Reference computation (N=8192 tokens, D=1024 in, O=1024 out, E=8 experts):
    gates = softmax(x @ Wg + bg)                        # [N, E]
    out   = sum_e gates[:, e] * (x @ We[e] + be[e])     # [N, O]

Strategy: data-parallel over tokens (1024 tokens/core, replicated weights),
with a mixed-precision decomposition that moves most of the matmul FLOPs to
fp8 DoubleRow (2 MACs/cell/cycle):

    out = x @ Wmean + g @ be                   (bf16 GEMMs, accurate)
        + sum_j c'_j * (x8 @ Wq8_j)            (7 fp8e4 DoubleRow GEMMs)

where Wmean = mean_e We and Wq_j = sum_e U[e,j] (We - Wmean) for U an [8,7]
orthonormal basis of the ones-perp subspace (Helmert).  Because
sum_e (We - Wmean) = 0, the 8 centered expert matrices span a rank-7 space,
so 7 fp8 GEMMs reproduce sum_e (g_e - 1/8)(x @ (We - Wmean)) EXACTLY in
infinite precision with coefficients c' = U.T (g - 1/8) = U.T g (U is
orthogonal to ones).  This saves 1/8 of the fp8 correction FLOPs; the
x-quantization noise folds back to the identical algebraic expression and
the W-quantization noise grows only 8/7 in variance.  Measured end-to-end
rel err ~1.83e-2 against the reference (gate 2e-2).

The PE streams 1 output column per cycle regardless of dtype (fp8 DR packs
2 contraction rows per column), so the kernel is a column-count problem:
  mean 8*8192 + bias 8*1024 + corrections 56*4096 + gating ~8k
  ~ 312k columns ~ 130us at 2.4 GHz.  Everything else must hide under it.

Measured DMA behaviour that shapes the schedule: each HWDGE ring delivers
its first bytes only ~3us after its dma_start executes (itself gated on
the framework preamble), then ramps to a few hundred GB/s -- and the two
rings share a limited early-bandwidth budget.  So the gating-critical x
tiles are interleaved across BOTH rings, the mean weights (k-split)
follow on the sync ring, and the 7MB of fp8 basis weights come last
(phase C consumes wq8[j] at ~49+14j us).  The PE burns the dead window on
a HAM warmup so the first real matmuls run at 2.4 GHz.

Per-core schedule:
  W : ~64 throwaway identity matmuls while the first x tiles land.
  G : gating logits TRANSPOSED per 4-tile half -- zT[e,n] = 8 N=512
      k-matmuls (amortizes LDWEIGHTS 4x vs per-tile); ACT adds bg while
      copying PSUM->SBUF.  Per tile: PE transposes the tile back, softmax
      on DVE/ACT, PE transposes g and computes the basis coefficients
      c' = g @ (U/2^16) as one tiny K=8 matmul.  These per-tile chains
      interleave with the first mean groups so their latency hides.
  B : per tile t, ONE [128,1024] two-bank PSUM group: 16 k-major bf16 mean
      matmuls (N=512 per bank) + the two K=8 bias matmuls LAST (so the
      group never waits on gating); one ACT copy drains it to acc.
  C : t-outer, j-inner: one [128,1024] PSUM group per (t,j) = 8 DR
      matmuls; a single fused DVE scalar_tensor_tensor folds it:
      acc += psum * c'[:, j].  Each tile's output streams out as soon as
      its 7 folds finish (split across both rings), so the 4MB of output
      spreads over the whole phase; the very last fold drains and DMAs
      per 512-col bank to shorten the tail.
"""

import numpy as np
import ml_dtypes

N_TOKENS = 8192
D = 1024   # in dim
O = 1024   # out dim
E = 8      # experts
NB = 7     # rank of the centered expert space
NCORES = 8
NLOC = N_TOKENS // NCORES   # 1024 tokens per core
KT = D // 128               # 8 k-chunks
TT = NLOC // 128            # 8 token chunks

BF16 = ml_dtypes.bfloat16
F8E4 = ml_dtypes.float8_e4m3   # IEEE e4m3: max normal 240, matches TRN fp8e4

XS = 32.0      # x fp8 scale (|x| < 5.2 -> < 166)
WS = 2048.0    # Wq fp8 scale (|Wq| < 0.054 -> < 111)
CINV = 1.0 / (XS * WS)

NWARM = 32     # HAM warm-up matmuls

_CACHE = {}


def _helmert_u():
    """[E, NB] orthonormal basis of the ones-perp subspace."""
    U = np.zeros((E, NB), dtype=np.float64)
    for j in range(1, E):
        U[:j, j - 1] = 1.0
        U[j, j - 1] = -j
        U[:, j - 1] /= np.sqrt(j * (j + 1))
    return U


def _build():
    """Build + compile the per-core Bass graph (same graph on all 8 cores)."""
    import concourse.bass as bass
    import concourse.mybir as mybir
    import concourse.tile as tile
    from concourse import bacc
    from concourse.masks import make_identity

    dt = mybir.dt
    f32 = dt.float32
    bf16 = dt.bfloat16
    f8e4 = dt.float8e4
    Alu = mybir.AluOpType
    DR = mybir.MatmulPerfMode.DoubleRow
    Act = mybir.ActivationFunctionType

    nc = bacc.Bacc(
        "TRN2",
        target_bir_lowering=False,
        debug=False,
        enable_asserts=False,
        num_devices=NCORES,
    )

    # t-major x: xt[p, t*D + k*128 + c] = x[t*128 + c, k*128 + p]
    xt_d = nc.dram_tensor("xt", [128, TT * D], bf16, kind="ExternalInput").ap()
    xt8_d = nc.dram_tensor("xt8", [128, TT * D], f8e4, kind="ExternalInput").ap()
    # k-major, j-contiguous weights: w[p, k*O + c] = W[k*128 + p, c]
    wm_d = nc.dram_tensor("Wmp", [128, KT * O], bf16, kind="ExternalInput").ap()
    wq8_d = nc.dram_tensor(
        "Wq8", [NB, 128, KT * O], f8e4, kind="ExternalInput"
    ).ap()
    be_d = nc.dram_tensor("bep", [E, O], bf16, kind="ExternalInput").ap()
    wg_d = nc.dram_tensor("Wgp", [128, KT * E], bf16, kind="ExternalInput").ap()
    bg_d = nc.dram_tensor("bgp", [E, 1], f32, kind="ExternalInput").ap()
    u_d = nc.dram_tensor("Up", [E, NB], bf16, kind="ExternalInput").ap()
    out_d = nc.dram_tensor("out", [NLOC, O], f32, kind="ExternalOutput").ap()

    with tile.TileContext(nc) as tc:
        with (
            tc.tile_pool(name="const", bufs=1) as cpool,
            tc.tile_pool(name="xp", bufs=1) as xpool,
            tc.tile_pool(name="wp", bufs=NB) as wpool,
            tc.tile_pool(name="ap", bufs=1) as apool,
            tc.tile_pool(name="gp", bufs=1) as gpool,
        ):
            ident = cpool.tile([128, 128], bf16)
            make_identity(nc, ident[:])

            # The two HWDGE rings share a limited early bandwidth budget
            # (~150GB/s each, ramping), so the gating-critical x tiles are
            # interleaved across BOTH rings; the mean weights follow on the
            # sync ring (needed from ~15us, k-progressive), then x8 and the
            # 7MB of fp8 basis weights (phase C needs wq8[j] at ~49+14j us).
            wg_sb = cpool.tile([128, KT * E], bf16)
            bg_sb = cpool.tile([E, 1], f32)
            u_sb = cpool.tile([E, NB], bf16)
            be_sb = cpool.tile([E, O], bf16)
            xt = xpool.tile([128, TT * D], bf16)
            wm = xpool.tile([128, KT, O], bf16)
            wm_dv = wm_d.rearrange("p (k c) -> p k c", k=KT)

            def dma_xt(eng, t):
                eng.dma_start(
                    xt[:, t * D : (t + 1) * D], xt_d[:, t * D : (t + 1) * D]
                )

            nc.scalar.dma_start(wg_sb[:], wg_d)
            dma_xt(nc.scalar, 0)
            nc.sync.dma_start(u_sb[:], u_d)
            dma_xt(nc.sync, 1)
            dma_xt(nc.scalar, 2)
            nc.sync.dma_start(wm[:, 0, :], wm_dv[:, 0])
            dma_xt(nc.sync, 3)
            nc.scalar.dma_start(bg_sb[:], bg_d)
            dma_xt(nc.scalar, 4)
            nc.sync.dma_start(wm[:, 1, :], wm_dv[:, 1])
            dma_xt(nc.sync, 5)
            dma_xt(nc.scalar, 6)
            nc.sync.dma_start(wm[:, 2, :], wm_dv[:, 2])
            dma_xt(nc.sync, 7)
            nc.sync.dma_start(wm[:, 3, :], wm_dv[:, 3])
            nc.sync.dma_start(wm[:, 4, :], wm_dv[:, 4])
            # be is only needed when mean group 0 closes (~+14us) -- keep it
            # behind the wm chunks that pace that group's k-loop
            nc.sync.dma_start(be_sb[:], be_d)
            for k in range(5, KT):
                nc.sync.dma_start(wm[:, k, :], wm_dv[:, k])
            xt8 = xpool.tile([128, TT, KT, 128], f8e4)
            nc.sync.dma_start(xt8[:].rearrange("p t k c -> p (t k c)"), xt8_d)
            wq8_tiles = []
            for j in range(NB):
                w8 = wpool.tile([128, KT, O], f8e4, tag="wq8", name=f"wq8_{j}")
                nc.sync.dma_start(w8[:].rearrange("p k c -> p (k c)"), wq8_d[j])
                wq8_tiles.append(w8)

            acc = apool.tile([128, TT * O], f32)

            zT_sb = gpool.tile([E, NLOC], bf16)
            g_f32 = gpool.tile([128, TT * E], f32)
            g_bf = gpool.tile([128, TT * E], bf16)
            gT = gpool.tile([E, NLOC], bf16)
            cc_sb = gpool.tile([128, TT * NB], f32)
            negm = gpool.tile([128, TT], f32)
            ssum = gpool.tile([128, TT], f32)
            rec = gpool.tile([128, TT], f32)

            def xt_tile(k, t):
                c = t * D + k * 128
                return xt[:, c : c + 128]

            # ---- Phase W: HAM warm-up on junk matmuls ---------------------
            with tc.tile_pool(name="psW", bufs=2, space="PSUM") as psW:
                wj = [
                    psW.tile([128, 128], f32, tag="wj", name=f"wj{i}")
                    for i in range(2)
                ]
                for i in range(NWARM):
                    nc.tensor.matmul(
                        wj[i % 2][:], ident[:], ident[:], start=True, stop=True
                    )

            xt_4d = xt.rearrange("p (t k c) -> p t k c", t=TT, k=KT)

            # ---- Phase G pieces: gating logits per 4-tile half ------------
            def zt_half(psZ, h):
                th = TT // 2
                ztp = psZ.tile([E, 512], f32, tag="zt")
                for k in range(KT):
                    nc.tensor.matmul(
                        ztp[:],
                        wg_sb[:, k * E : (k + 1) * E],
                        xt_4d[:, h * th : (h + 1) * th, k, :],
                        start=(k == 0),
                        stop=(k == KT - 1),
                    )
                # + bg (per-partition bias) while copying PSUM -> SBUF
                nc.scalar.activation(
                    zT_sb[:, h * 512 : (h + 1) * 512],
                    ztp[:],
                    Act.Identity,
                    bias=bg_sb[:, 0:1],
                    scale=1.0,
                )

            def zg_tr(psA, t):
                zg = psA.tile([128, E], bf16, tag="zg")
                nc.tensor.transpose(
                    zg[:], zT_sb[:, t * 128 : (t + 1) * 128], ident[:E, :E]
                )
                nm = negm[:, t : t + 1]
                nc.vector.tensor_reduce(
                    nm, zg[:], axis=mybir.AxisListType.X, op=Alu.max, negate=True
                )
                gs = g_f32[:, t * E : (t + 1) * E]
                nc.scalar.activation(
                    gs,
                    zg[:],
                    Act.Exp,
                    bias=nm,
                    scale=1.0,
                    accum_out=ssum[:, t : t + 1],
                )
                nc.vector.reciprocal(rec[:, t : t + 1], ssum[:, t : t + 1])
                nc.vector.tensor_scalar_mul(gs, gs, rec[:, t : t + 1])
                nc.vector.tensor_copy(g_bf[:, t * E : (t + 1) * E], gs)

            def g_tr(psC, psD, t):
                trp = psC.tile([E, 128], bf16, tag="tr")
                nc.tensor.transpose(
                    trp[:], g_bf[:, t * E : (t + 1) * E], ident[:]
                )
                nc.vector.tensor_copy(gT[:, t * 128 : (t + 1) * 128], trp[:])
                # c' = g @ (U / 2^16): centered basis coefficients with the
                # fp8 scale factors XS*WS divided out (U is ones-perp, so
                # the -1/8 centering drops out exactly).
                ccp = psD.tile([128, NB], f32, tag="cc")
                nc.tensor.matmul(
                    ccp[:],
                    gT[:, t * 128 : (t + 1) * 128],
                    u_sb[:],
                    start=True,
                    stop=True,
                )
                nc.scalar.activation(
                    cc_sb[:, t * NB : (t + 1) * NB], ccp[:], Act.Copy
                )

            # ---- Phase B: mean + bias, one PSUM group per token tile ------
            # k-major so the group is paced by the wm[k] DMA arrivals; the
            # two K=8 bias matmuls close the banks LAST so the group never
            # stalls on the gating chain.
            def mean_group(psM, t):
                pm = psM.tile([128, O], f32, tag="pm")
                for k in range(KT):
                    for jj in range(2):
                        sl = slice(jj * 512, (jj + 1) * 512)
                        nc.tensor.matmul(
                            pm[:, sl],
                            xt_tile(k, t),
                            wm[:, k, sl],
                            start=(k == 0),
                            stop=False,
                        )
                for jj in range(2):
                    sl = slice(jj * 512, (jj + 1) * 512)
                    nc.tensor.matmul(
                        pm[:, sl],
                        gT[:, t * 128 : (t + 1) * 128],
                        be_sb[:, sl],
                        start=False,
                        stop=True,
                    )
                nc.scalar.activation(acc[:, t * O : (t + 1) * O], pm[:], Act.Copy)

            # zT halves run as soon as x tiles 0-3 / 4-7 land (~14/17us);
            # the per-tile softmax chain and the mean groups interleave
            # behind them so softmax latency hides under mean streaming.
            with (
                tc.tile_pool(name="psZ", bufs=1, space="PSUM") as psZ,
                tc.tile_pool(name="psA", bufs=1, space="PSUM") as psA,
                tc.tile_pool(name="psC", bufs=1, space="PSUM") as psC,
                tc.tile_pool(name="psD", bufs=1, space="PSUM") as psD,
                tc.tile_pool(name="psM", bufs=2, space="PSUM") as psM,
            ):
                zt_half(psZ, 0)
                zg_tr(psA, 0)
                zg_tr(psA, 1)
                zg_tr(psA, 2)
                zg_tr(psA, 3)
                zt_half(psZ, 1)
                g_tr(psC, psD, 0)
                g_tr(psC, psD, 1)
                zg_tr(psA, 4)
                g_tr(psC, psD, 2)
                zg_tr(psA, 5)
                g_tr(psC, psD, 3)
                mean_group(psM, 0)
                zg_tr(psA, 6)
                g_tr(psC, psD, 4)
                mean_group(psM, 1)
                zg_tr(psA, 7)
                g_tr(psC, psD, 5)
                mean_group(psM, 2)
                g_tr(psC, psD, 6)
                mean_group(psM, 3)
                g_tr(psC, psD, 7)
                for t in range(4, TT):
                    mean_group(psM, t)

            # ---- Phase C: fp8 DoubleRow basis corrections -----------------
            # One [128,1024] two-bank PSUM group per (t,j) = 8 DR matmuls;
            # a single fused STT folds it into acc with the per-token
            # coefficient c'[:, j].  t-OUTER so each tile's output streams
            # as soon as its 7 folds are done -- the 4MB of output spreads
            # over the whole ~97us phase instead of jamming the rings in
            # the last basis pass (phase C starts ~60us; all wq8 land by
            # ~45us, so consuming every basis per tile is safe).
            with tc.tile_pool(name="psB", bufs=4, space="PSUM") as psB:
                for t in range(TT):
                    for j in range(NB):
                        w8 = wq8_tiles[j]
                        ps = psB.tile([128, O], f32, tag="mm")
                        for jj in range(2):
                            sl = slice(jj * 512, (jj + 1) * 512)
                            for k2 in range(KT // 2):
                                nc.tensor.matmul(
                                    ps[:, sl],
                                    xt8[:, t, 2 * k2 : 2 * k2 + 2, :],
                                    w8[:, 2 * k2 : 2 * k2 + 2, sl],
                                    start=(k2 == 0),
                                    stop=(k2 == KT // 2 - 1),
                                    perf_mode=DR,
                                )
                        cc = cc_sb[:, t * NB + j : t * NB + j + 1]
                        if t == TT - 1 and j == NB - 1:
                            # last fold: drain + DMA per 512-col bank so the
                            # final output transfer starts one bank early
                            for jj, eng in ((0, nc.sync), (1, nc.scalar)):
                                sl = slice(jj * 512, (jj + 1) * 512)
                                a_sl = acc[:, t * O + jj * 512 : t * O + (jj + 1) * 512]
                                nc.vector.scalar_tensor_tensor(
                                    a_sl, ps[:, sl], cc, a_sl,
                                    op0=Alu.mult, op1=Alu.add,
                                )
                                eng.dma_start(
                                    out_d[t * 128 : (t + 1) * 128, sl], a_sl
                                )
                        else:
                            a_sl = acc[:, t * O : (t + 1) * O]
                            nc.vector.scalar_tensor_tensor(
                                a_sl, ps[:], cc, a_sl, op0=Alu.mult, op1=Alu.add,
                            )
                    if t < TT - 1:
                        # split each tile's output across both rings
                        rows = slice(t * 128, (t + 1) * 128)
                        nc.sync.dma_start(
                            out_d[rows, 0:512], acc[:, t * O : t * O + 512]
                        )
                        nc.scalar.dma_start(
                            out_d[rows, 512:1024],
                            acc[:, t * O + 512 : (t + 1) * O],
                        )

    nc.compile()
    return nc


def _get_nc():
    if "nc" not in _CACHE:
        _CACHE["nc"] = _build()
    return _CACHE["nc"]


def _pack_inputs(x, We, be, Wg, bg):
    """Host-side packing: shard + pre-transpose + cast to bf16/fp8."""
    x = np.asarray(x, dtype=np.float32)
    We = np.asarray(We, dtype=np.float32)
    be = np.asarray(be, dtype=np.float32)
    Wg = np.asarray(Wg, dtype=np.float32)
    bg = np.asarray(bg, dtype=np.float32)

    Wmean = We.mean(axis=0)
    Wp = We - Wmean[None]
    U = _helmert_u()
    Wq = np.einsum("ej,eio->jio", U, Wp).astype(np.float32)

    def ptrans_k(w):  # [D, O] -> [128, KT*O], [p, k, c] = w[k*128 + p, c]
        return np.ascontiguousarray(
            w.reshape(KT, 128, O).transpose(1, 0, 2).reshape(128, -1)
        )

    wm_p = ptrans_k(Wmean).astype(BF16)
    wq8_p = np.stack(
        [np.clip(ptrans_k(Wq[j]) * WS, -240, 240).astype(F8E4) for j in range(NB)]
    )
    be_p = be.astype(BF16)
    wg_p = np.ascontiguousarray(
        Wg.reshape(KT, 128, E).transpose(1, 0, 2).reshape(128, KT * E)
    ).astype(BF16)
    bg_p = bg.reshape(E, 1).astype(np.float32)
    u_p = (U * CINV).astype(BF16)

    in_maps = []
    for i in range(NCORES):
        xs = x[i * NLOC : (i + 1) * NLOC]          # [NLOC, D]
        # xt[p, t*D + k*128 + c] = xs[t*128+c, k*128+p]
        xt_f = np.ascontiguousarray(
            xs.T.reshape(KT, 128, TT, 128).transpose(1, 2, 0, 3).reshape(128, TT * D)
        )
        xt = xt_f.astype(BF16)
        xt8 = np.clip(xt_f * XS, -240, 240).astype(F8E4)
        in_maps.append(
            {
                "xt": xt,
                "xt8": xt8,
                "Wmp": wm_p,
                "Wq8": wq8_p,
                "bep": be_p,
                "Wgp": wg_p,
                "bgp": bg_p,
                "Up": u_p,
            }
        )
    return in_maps


def _run(inputs, trace=False):
    """Returns (y_full, BassKernelResults)."""
    from concourse.bass_utils import run_bass_kernel_spmd

    nc = _get_nc()
    in_maps = _pack_inputs(**inputs)
    res = run_bass_kernel_spmd(
        nc, in_maps, core_ids=list(range(NCORES)), trace=trace
    )
    y = np.concatenate(
        [res.results[i]["out"] for i in range(NCORES)], axis=0
    ).astype(np.float32)
    return y, res


def kernel(**inputs):
    y, _ = _run(inputs, trace=False)
    return y
